# revision 33
# baseline (speedup 1.0000x reference)
"""Trainium2 Bass kernel for the DiffSSM block.

Data-parallel over batch B=8 across 8 NeuronCores (one batch element per
core). All heavy compute runs on the TensorEngine in bf16 with fp32 PSUM
accumulation; the tiny SSM kernel generation, timestep embedding, and
Toeplitz construction are host-side precompute.

v2 engine-balance redesign (vs the phase-serial baseline):
  - Bias adds folded into the matmul accumulation groups as K=1 matmuls
    (ones x bias_row), so LN stats run directly on PSUM.
  - LN normalize (x*rstd - mean*rstd) moved to ScalarE activation with
    per-partition scale/bias APs; only the g/b affine stays on VectorE.
  - Phase B (Toeplitz mix) evicts through ScalarE (Copy, scale=noise),
    phase E eviction stays on VectorE (scalar_tensor_tensor accumulate).
  - hln -> hlnT transpose done as 32 strip-wise xbar DMA transposes
    (per 512-row strip x 128-col block), overlapped with phase A.
  - Conv loops run ot-outer with double-buffered per-ot weight chunks
    (12 KB resident instead of 48 KB), double-buffered TmT chunks, and
    strip-buffered xT loads, so every phase's operands prefetch during
    the previous phase within the SBUF budget.

Device phases: A proj-in+LN1 -> B Toeplitz mix -> D conv1+Silu ->
E conv2 accumulate -> F proj-out+LN2+residual.
"""

import math

import numpy as np
import ml_dtypes

_BF16 = ml_dtypes.bfloat16

_L, _D, _B = 2048, 1024, 8

_cache = {}


def _build(L, D, n_cores):
    import concourse.bacc as bacc
    import concourse.bass as bass
    import concourse.tile as tile
    from concourse import mybir

    f32 = mybir.dt.float32
    bf16 = mybir.dt.bfloat16
    AF = mybir.ActivationFunctionType
    OP = mybir.AluOpType

    P = 128
    KT = D // P            # feature tiles
    LT = L // P            # sequence tiles
    ND = min(512, D)       # matmul free-dim chunk along features
    NF = min(512, L)       # matmul free-dim chunk along sequence
    EH = D // ND
    LC = L // NF
    ST = LT
    XSW = 256              # xT strip width
    SPL = XSW // P         # lt tiles per xT strip (2)
    TSW = 512              # transpose strip width (xbar free-dim mult 128)
    TPL = TSW // P         # lt tiles per transpose strip

    nc = bacc.Bacc("TRN2", target_bir_lowering=False, debug=False,
                   num_devices=n_cores)

    x_res = nc.dram_tensor("x_res", (L, D), f32, kind="ExternalInput").ap()
    xT = nc.dram_tensor("xT", (D, L), bf16, kind="ExternalInput").ap()
    Wi = nc.dram_tensor("Wi", (D, D), bf16, kind="ExternalInput").ap()
    w1R = nc.dram_tensor("w1R", (KT, P, KT, 3, P), bf16,
                         kind="ExternalInput").ap()
    w2R = nc.dram_tensor("w2R", (KT, P, KT, 3, P), bf16,
                         kind="ExternalInput").ap()
    Wo = nc.dram_tensor("Wo", (D, D), bf16, kind="ExternalInput").ap()
    NS = 72                # SSM states (64 modes + Df/Db const + pad)
    QC = 512               # SSD chunk length
    NCH = L // QC          # chunks
    SPC = QC // 128        # 128-tiles per chunk
    TmD = nc.dram_tensor("TmD", (1, 128, SPC, QC), bf16,
                         kind="ExternalInput").ap()
    AfP = nc.dram_tensor("AfP", (128, SPC, NS), bf16,
                         kind="ExternalInput").ap()
    AbP = nc.dram_tensor("AbP", (128, SPC, NS), bf16,
                         kind="ExternalInput").ap()
    CfO = nc.dram_tensor("CfO", (NS, QC), bf16, kind="ExternalInput").ap()
    CbO = nc.dram_tensor("CbO", (NS, QC), bf16, kind="ExternalInput").ap()
    lamf = nc.dram_tensor("lamf", (NS, 1), f32, kind="ExternalInput").ap()
    lamb = nc.dram_tensor("lamb", (NS, 1), f32, kind="ExternalInput").ap()
    nsc = nc.dram_tensor("nsc", (P, KT), f32, kind="ExternalInput").ap()
    bc1c = nc.dram_tensor("bc1c", (P, KT), f32, kind="ExternalInput").ap()
    bc2c = nc.dram_tensor("bc2c", (P, KT), f32, kind="ExternalInput").ap()
    bi_row = nc.dram_tensor("bi_row", (1, D), bf16, kind="ExternalInput").ap()
    bo_row = nc.dram_tensor("bo_row", (1, D), bf16, kind="ExternalInput").ap()
    vec_names = ["g1v", "b1v", "g2v"]
    vecs = {n: nc.dram_tensor(n, (D,), bf16, kind="ExternalInput").ap()
            for n in vec_names}
    out = nc.dram_tensor("out", (L, D), f32, kind="ExternalOutput").ap()

    with tile.TileContext(nc) as tc:
        # ---- pools (left stack, release order = reverse alloc) ----
        const = tc.alloc_tile_pool(name="const", bufs=1)
        statp = tc.alloc_tile_pool(name="stat", bufs=4)
        psum = tc.alloc_tile_pool(name="psum", bufs=8, space="PSUM")
        h2T_pool = tc.alloc_tile_pool(name="h2T", bufs=1)
        w2ch_pool = tc.alloc_tile_pool(name="w2ch", bufs=2)
        hlnT_pool = tc.alloc_tile_pool(name="hlnT", bufs=1)
        w1ch_pool = tc.alloc_tile_pool(name="w1ch", bufs=2)
        tb_pool = tc.alloc_tile_pool(name="tb", bufs=2)
        pa_pool = tc.alloc_tile_pool(name="pa", bufs=1)
        # right stack: hln (released end of B), then Fpool, co
        hln_pool = tc.alloc_tile_pool(name="hln", bufs=1, side="right")

        # ---- constants ----
        def rep_tile(name, pool=None):
            t = (pool or const).tile([P, D], bf16, tag=name,
                                     name=f"rep_{name}")
            ap = vecs[name]
            bcast = bass.AP(tensor=ap.tensor, offset=ap.offset,
                            ap=[[0, P]] + list(ap.ap))
            nc.gpsimd.dma_start(out=t[:], in_=bcast)
            return t

        bi_sb = const.tile([1, D], bf16)
        nc.sync.dma_start(out=bi_sb[:], in_=bi_row)
        eps_sb = const.tile([P, 1], f32)
        nc.vector.memset(eps_sb[:], 1e-5)
        ones_sb = const.tile([1, P], bf16)
        nc.vector.memset(ones_sb[:], 1.0)
        # preload the (large) sqrt activation table while initial DMAs run
        warm_sb = const.tile([P, 1], f32)
        nc.scalar.activation(out=warm_sb[:], in_=eps_sb[:], func=AF.Sqrt)
        ns_sb = const.tile([P, KT], f32)
        nc.sync.dma_start(out=ns_sb[:], in_=nsc)
        bc1_sb = const.tile([P, KT], f32)
        nc.sync.dma_start(out=bc1_sb[:], in_=bc1c)
        bc2_sb = const.tile([P, KT], f32)
        nc.sync.dma_start(out=bc2_sb[:], in_=bc2c)
        AfP_sb = const.tile([P, SPC, NS], bf16)
        nc.sync.dma_start(out=AfP_sb[:], in_=AfP)
        AbP_sb = const.tile([P, SPC, NS], bf16)
        nc.sync.dma_start(out=AbP_sb[:], in_=AbP)
        CfO_sb = const.tile([NS, QC], bf16)
        nc.sync.dma_start(out=CfO_sb[:], in_=CfO)
        CbO_sb = const.tile([NS, QC], bf16)
        nc.sync.dma_start(out=CbO_sb[:], in_=CbO)
        lamf_sb = const.tile([NS, 1], f32)
        nc.sync.dma_start(out=lamf_sb[:], in_=lamf)
        lamb_sb = const.tile([NS, 1], f32)
        nc.sync.dma_start(out=lamb_sb[:], in_=lamb)

        h2T_sb = h2T_pool.tile([P, KT, L], bf16)
        hlnT_sb = hlnT_pool.tile([P, KT, L], bf16)
        hln_sb = hln_pool.tile([P, LT, D], bf16)

        wi_sb = pa_pool.tile([P, KT, D], bf16, tag="wi")
        wi_r = Wi.rearrange("(kt p) d -> p kt d", p=P)
        xT_r = xT.rearrange("(kt p) l -> p kt l", p=P)

        def ln_scalars(stats_tile):
            """stats -> (mv, rstd) tiles."""
            mv = statp.tile([P, 2], f32, tag="mv", name="mv")
            nc.vector.bn_aggr(out=mv[:], in_=stats_tile[:])
            std = statp.tile([P, 1], f32, tag="std", name="std")
            nc.scalar.activation(out=std[:], in_=mv[:, 1:2], func=AF.Sqrt,
                                 bias=eps_sb[:], scale=1.0)
            rstd = statp.tile([P, 1], f32, tag="rstd", name="rstd")
            nc.vector.reciprocal(out=rstd[:], in_=std[:])
            return mv, rstd

        # ---- Phase A: proj-in + LN1 (stats on PSUM, norm on ScalarE) ----
        xs = None
        for lt in range(LT):
            ls = lt // SPL
            if lt == 0:
                nc.gpsimd.dma_start(out=wi_sb[:, :, 0:ND],
                                    in_=wi_r[:, :, 0:ND])
            if lt % SPL == 0:
                xs = pa_pool.tile([P, KT, XSW], bf16, tag="xs", name="xs",
                                  bufs=4)
                nc.gpsimd.dma_start(
                    out=xs[:],
                    in_=xT_r[:, :, ls * XSW:(ls + 1) * XSW])
            if lt == 0:
                nc.gpsimd.dma_start(out=wi_sb[:, :, ND:D],
                                    in_=wi_r[:, :, ND:D])
                g1_rep = rep_tile("g1v")
                b1_rep = rep_tile("b1v")
            if lt == 2:
                # the Toeplitz diag block is chunk-invariant: load it once
                Tc = tb_pool.tile([P, SPC, QC], bf16, tag="Tc", name="Tc",
                                  bufs=1)
                nc.gpsimd.dma_start(out=Tc[:], in_=TmD[0])
            col = (lt % SPL) * P
            stats = statp.tile([P, EH, 6], f32, tag="stats", name="stats")
            nrm = statp.tile([P, D], bf16, tag="nrm", name="nrm", bufs=8)
            for eh in range(EH):
                ps = psum.tile([P, ND], f32, tag="ps", name="ps")
                nc.tensor.matmul(ps[:], lhsT=ones_sb[:],
                                 rhs=bi_sb[:, eh * ND:(eh + 1) * ND],
                                 start=True, stop=False)
                for kt in range(KT):
                    nc.tensor.matmul(ps[:],
                                     lhsT=xs[:, kt, col:col + P],
                                     rhs=wi_sb[:, kt, eh * ND:(eh + 1) * ND],
                                     start=False, stop=(kt == KT - 1))
                nc.scalar.activation(out=nrm[:, eh * ND:(eh + 1) * ND],
                                     in_=ps[:], func=AF.Copy)
                nc.vector.bn_stats(out=stats[:, eh, :],
                                   in_=nrm[:, eh * ND:(eh + 1) * ND])
            mv, rstd = ln_scalars(stats)
            nc.vector.tensor_scalar(out=nrm[:], in0=nrm[:],
                                    scalar1=mv[:, 0:1], scalar2=rstd[:],
                                    op0=OP.subtract, op1=OP.mult)
            nc.vector.tensor_mul(out=nrm[:], in0=nrm[:], in1=g1_rep[:])
            nc.vector.tensor_add(out=hln_sb[:, lt, :], in0=nrm[:],
                                 in1=b1_rep[:])
        # hln -> hlnT SBUF->SBUF xbar transposes, one per lt tile. The
        # xbar-mode switch serializes against ALL in-flight DMA copies, so
        # a transpose scheduled mid-A stalls the ACT sequencer for ~10us.
        # Guard: write one byte into every transpose's output block, with
        # the guard reading A's last hln tile -- every transpose then
        # WAW-depends on A being fully done and the batch runs back-to-back
        # at B's start (gating only phase D).
        hlnT_r = hlnT_sb[:].rearrange("p kt (lt c) -> p kt lt c", c=P)
        nc.scalar.activation(out=hlnT_r[:, 0, :, 0:1],
                             in_=hln_sb[:, LT - 1, 0:LT], func=AF.Copy)
        for lt in range(LT):
            nc.scalar.dma_start_transpose(
                out=hlnT_sb[:, :, lt * P:(lt + 1) * P],
                in_=hln_sb[:, lt, :])

        # ---- Phase B: SSD chunked SSM mix ----
        # cross-chunk states: Zf[c] = sum_{cs<c} Lam^(Q(c-cs-1)) Pf[cs],
        # Gb[c] = sum_{cs>c} Lam^(Q(cs-c-1)) Pb[cs]; recurrences fused into
        # the PSUM evictions (scalar_tensor_tensor).
        def state_proj(proj_sb, c):
            pss = []
            for eh in range(EH):
                ps = psum.tile([NS, ND], f32, tag="ps", name="psP")
                for st in range(SPC):
                    nc.tensor.matmul(
                        ps[:], lhsT=proj_sb[:, st, :],
                        rhs=hln_sb[:, c * SPC + st, eh * ND:(eh + 1) * ND],
                        start=(st == 0), stop=(st == SPC - 1))
                pss.append(ps)
            return pss

        def state_tile(name):
            return tb_pool.tile([NS, D], bf16, tag=name, name=name, bufs=1)

        Zf = {}
        for c in range(NCH - 1):
            pss = state_proj(AfP_sb, c)
            Zf[c + 1] = state_tile(f"Zf{c + 1}")
            for eh in range(EH):
                sl = slice(eh * ND, (eh + 1) * ND)
                if c == 0:
                    nc.vector.tensor_copy(out=Zf[1][:, sl],
                                          in_=pss[eh][:])
                else:
                    nc.vector.scalar_tensor_tensor(
                        out=Zf[c + 1][:, sl], in0=Zf[c][:, sl],
                        scalar=lamf_sb[:, 0:1], in1=pss[eh][:],
                        op0=OP.mult, op1=OP.add)
        Gb = {}
        for c in range(NCH - 1, 0, -1):
            pss = state_proj(AbP_sb, c)
            Gb[c - 1] = state_tile(f"Gb{c - 1}")
            for eh in range(EH):
                sl = slice(eh * ND, (eh + 1) * ND)
                if c == NCH - 1:
                    nc.vector.tensor_copy(out=Gb[c - 1][:, sl],
                                          in_=pss[eh][:])
                else:
                    nc.vector.scalar_tensor_tensor(
                        out=Gb[c - 1][:, sl], in0=Gb[c][:, sl],
                        scalar=lamb_sb[:, 0:1], in1=pss[eh][:],
                        op0=OP.mult, op1=OP.add)

        w1_next = None
        for tch in range(NCH):
            if tch == 0:
                # prefetch first conv1 weight chunk during B
                w1_next = w1ch_pool.tile([P, KT, 3, P], bf16, tag="w1c",
                                         name="w1c")
                nc.gpsimd.dma_start(out=w1_next[:], in_=w1R[0])
            for dt in range(KT):
                ps = psum.tile([P, QC], f32, tag="ps", name="ps")
                n_mm = SPC + (tch > 0) + (tch < NCH - 1)
                k = 0
                for st in range(SPC):
                    k += 1
                    nc.tensor.matmul(
                        ps[:],
                        lhsT=hln_sb[:, tch * SPC + st, dt * P:(dt + 1) * P],
                        rhs=Tc[:, st, :],
                        start=(st == 0), stop=(k == n_mm))
                if tch > 0:
                    k += 1
                    nc.tensor.matmul(ps[:],
                                     lhsT=Zf[tch][:, dt * P:(dt + 1) * P],
                                     rhs=CfO_sb[:], start=False,
                                     stop=(k == n_mm))
                if tch < NCH - 1:
                    k += 1
                    nc.tensor.matmul(ps[:],
                                     lhsT=Gb[tch][:, dt * P:(dt + 1) * P],
                                     rhs=CbO_sb[:], start=False,
                                     stop=(k == n_mm))
                nc.vector.tensor_scalar_mul(
                    out=h2T_sb[:, dt, tch * QC:(tch + 1) * QC],
                    in0=ps[:], scalar1=ns_sb[:, dt:dt + 1])
        pa_pool.release()
        tb_pool.release()
        hln_pool.release()

        # Fpool + co on the (now empty) right stack; loads overlap D/E.
        Fpool = tc.alloc_tile_pool(name="Fp", bufs=1, side="right")
        co_pool = tc.alloc_tile_pool(name="co", bufs=1, side="right")
        wo_sb = Fpool.tile([P, KT, D], bf16, tag="wo")
        wo_r = Wo.rearrange("(dt p) e -> dt p e", p=P)
        for dt in range(KT):
            nc.gpsimd.dma_start(out=wo_sb[:, dt, :], in_=wo_r[dt])
        bo_sb = Fpool.tile([1, D], bf16, tag="bo")
        nc.sync.dma_start(out=bo_sb[:], in_=bo_row)
        g2_rep = rep_tile("g2v", pool=Fpool)
        co_sb = co_pool.tile([P, KT, L], bf16)

        def conv_mms(ps, w_t, src_sb, lc):
            # kernel-3 conv as 3 shifted matmuls; j=1 (no shift) first so
            # start=True initializes the whole PSUM range; border columns
            # handled by narrowing the edge matmuls.
            first = True
            for it in range(KT):
                for j in (1, 0, 2):
                    o0 = 1 if (j == 0 and lc == 0) else 0
                    o1 = NF - 1 if (j == 2 and lc == LC - 1) else NF
                    base = lc * NF + j - 1
                    nc.tensor.matmul(
                        ps[:, o0:o1],
                        lhsT=w_t[:, it, j, :],
                        rhs=src_sb[:, it, base + o0:base + o1],
                        start=first,
                        stop=(it == KT - 1 and j == 2))
                    first = False

        # ---- Phase D: conv1 (+Silu), ot-outer with weight streaming ----
        w2_next = None
        for ot in range(KT):
            w1t = w1_next
            if ot + 1 < KT:
                w1_next = w1ch_pool.tile([P, KT, 3, P], bf16, tag="w1c",
                                         name="w1c")
                nc.gpsimd.dma_start(out=w1_next[:], in_=w1R[ot + 1])
            if ot == 0:
                w2_next = w2ch_pool.tile([P, KT, 3, P], bf16, tag="w2c",
                                         name="w2c")
                nc.gpsimd.dma_start(out=w2_next[:], in_=w2R[0])
            for lc in range(LC):
                ps = psum.tile([P, NF], f32, tag="ps", name="ps")
                conv_mms(ps, w1t, hlnT_sb, lc)
                nc.scalar.activation(
                    out=co_sb[:, ot, lc * NF:(lc + 1) * NF],
                    in_=ps[:], func=AF.Silu, bias=bc1_sb[:, ot:ot + 1],
                    scale=1.0)
        w1ch_pool.release()
        hlnT_pool.release()

        # ---- Phase E: conv2, accumulate into h2T ----
        for ot in range(KT):
            w2t = w2_next
            if ot + 1 < KT:
                w2_next = w2ch_pool.tile([P, KT, 3, P], bf16, tag="w2c",
                                         name="w2c")
                nc.gpsimd.dma_start(out=w2_next[:], in_=w2R[ot + 1])
            for lc in range(LC):
                ps = psum.tile([P, NF], f32, tag="ps", name="ps")
                conv_mms(ps, w2t, co_sb, lc)
                nc.vector.scalar_tensor_tensor(
                    out=h2T_sb[:, ot, lc * NF:(lc + 1) * NF],
                    in0=ps[:], scalar=bc2_sb[:, ot:ot + 1],
                    in1=h2T_sb[:, ot, lc * NF:(lc + 1) * NF],
                    op0=OP.add, op1=OP.add)
        w2ch_pool.release()
        co_pool.release()

        # ---- Phase F: proj-out + LN2 + residual ----
        x_r = x_res.rearrange("(t p) d -> t p d", p=P)
        out_r = out.rearrange("(t p) d -> t p d", p=P)
        for lt in range(LT):
            x_t = Fpool.tile([P, D], f32, tag="x_t", name="x_t", bufs=2)
            nc.sync.dma_start(out=x_t[:], in_=x_r[lt])
            stats = statp.tile([P, EH, 6], f32, tag="stats", name="stats")
            y = Fpool.tile([P, D], bf16, tag="y", name="y", bufs=4)
            for eh in range(EH):
                ps = psum.tile([P, ND], f32, tag="ps", name="ps")
                nc.tensor.matmul(ps[:], lhsT=ones_sb[:],
                                 rhs=bo_sb[:, eh * ND:(eh + 1) * ND],
                                 start=True, stop=False)
                for dt in range(KT):
                    nc.tensor.matmul(ps[:],
                                     lhsT=h2T_sb[:, dt, lt * P:(lt + 1) * P],
                                     rhs=wo_sb[:, dt, eh * ND:(eh + 1) * ND],
                                     start=False, stop=(dt == KT - 1))
                nc.scalar.activation(out=y[:, eh * ND:(eh + 1) * ND],
                                     in_=ps[:], func=AF.Copy)
                nc.vector.bn_stats(out=stats[:, eh, :],
                                   in_=y[:, eh * ND:(eh + 1) * ND])
            mv, rstd = ln_scalars(stats)
            nc.vector.tensor_scalar(out=y[:], in0=y[:],
                                    scalar1=mv[:, 0:1], scalar2=rstd[:],
                                    op0=OP.subtract, op1=OP.mult)
            nc.vector.tensor_mul(out=y[:], in0=y[:], in1=g2_rep[:])
            nc.vector.tensor_add(out=x_t[:], in0=y[:], in1=x_t[:])
            nc.sync.dma_start(out=out_r[lt], in_=x_t[:])
        h2T_pool.release()
        psum.release()
        statp.release()
        const.release()
        Fpool.release()

    nc.compile()
    return nc


def _bf(a):
    return np.ascontiguousarray(np.asarray(a, np.float32)).astype(_BF16)


def _prep_maps(inputs, L, D, n_cores):
    P = 128
    KT = D // P
    f32 = np.float32
    x = np.asarray(inputs["x"], f32)
    t = np.asarray(inputs["t"], f32)
    beta1 = float(np.asarray(inputs["beta1"], f32)[0])
    beta2 = float(np.asarray(inputs["beta2"], f32)[0])

    # SSM kernels -> mixed Toeplitz (transposed), host fp32
    af = np.diagonal(np.asarray(inputs["Af"], f32))
    ab = np.diagonal(np.asarray(inputs["Ab"], f32))
    l_ar = np.arange(L, dtype=f32)[:, None]
    kf = np.exp(l_ar * af[None, :]) @ (
        np.asarray(inputs["Bf"], f32)[:, 0] * np.asarray(inputs["Cf"], f32)[0]
    ) + np.asarray(inputs["Df"], f32)[0]
    kb = np.exp(l_ar * ab[None, :]) @ (
        np.asarray(inputs["Bb"], f32)[:, 0] * np.asarray(inputs["Cb"], f32)[0]
    ) + np.asarray(inputs["Db"], f32)[0]
    # within-chunk mixed Toeplitz diagonal blocks (exact)
    QC, NS = 512, 72
    NCH = L // QC
    tms = np.arange(QC)[None, :] - np.arange(QC)[:, None]  # [s_loc, t_loc]
    TmQ = (np.where(tms >= 0, beta1 * kf[np.clip(tms, 0, None)], 0.0)
           + np.where(tms <= 0, beta2 * kb[np.clip(-tms, 0, None)], 0.0))
    TmD = np.ascontiguousarray(
        TmQ.reshape(1, QC // 128, 128, QC).transpose(0, 2, 1, 3)
    ).astype(f32).astype(_BF16)
    # cross-chunk rank-NS state matrices (64 modes + const Df/Db state)
    wf = (np.asarray(inputs["Bf"], f32)[:, 0]
          * np.asarray(inputs["Cf"], f32)[0])
    wb = (np.asarray(inputs["Bb"], f32)[:, 0]
          * np.asarray(inputs["Cb"], f32)[0])
    Df = float(np.asarray(inputs["Df"], f32)[0])
    Db = float(np.asarray(inputs["Db"], f32)[0])
    s_loc = np.arange(QC, dtype=f32)
    AfP = np.zeros((QC, NS), f32)
    AfP[:, :64] = np.exp((QC - 1 - s_loc)[:, None] * af[None, :])
    AfP[:, 64] = 1.0
    AbP = np.zeros((QC, NS), f32)
    AbP[:, :64] = np.exp((s_loc + 1)[:, None] * ab[None, :])
    AbP[:, 64] = 1.0
    t_loc = np.arange(QC, dtype=f32)
    CfO = np.zeros((NS, QC), f32)
    CfO[:64] = beta1 * wf[:, None] * np.exp(af[:, None] * (t_loc + 1)[None])
    CfO[64] = beta1 * Df
    CbO = np.zeros((NS, QC), f32)
    CbO[:64] = beta2 * wb[:, None] * np.exp(
        ab[:, None] * (QC - 1 - t_loc)[None])
    CbO[64] = beta2 * Db
    lamf = np.zeros((NS, 1), f32)
    lamf[:64, 0] = np.exp(af * QC)
    lamf[64, 0] = 1.0
    lamb = np.zeros((NS, 1), f32)
    lamb[:64, 0] = np.exp(ab * QC)
    lamb[64, 0] = 1.0
    AfP_d = np.ascontiguousarray(
        AfP.reshape(QC // 128, 128, NS).transpose(1, 0, 2)).astype(_BF16)
    AbP_d = np.ascontiguousarray(
        AbP.reshape(QC // 128, 128, NS).transpose(1, 0, 2)).astype(_BF16)

    # timestep embedding -> noise scale (B, D)
    half = D // 2
    freqs = np.exp(np.arange(half, dtype=f32)
                   * (-math.log(10000.0) / (half - 1)))
    ang = t[:, None] * freqs[None, :]
    emb = np.concatenate([np.sin(ang), np.cos(ang)], axis=1).astype(f32)
    ns = (1.0 / (1.0 + np.exp(-emb))).astype(f32)         # (B, D)

    Wi_bf = _bf(inputs["Wi"])
    Wo_bf = _bf(inputs["Wo"])

    def conv_w(w):
        # (D_o, D_i, 3) -> [ot, p_i, it, j, o_local]
        w = np.asarray(w, f32).reshape(KT, P, KT, P, 3)
        return np.ascontiguousarray(
            w.transpose(0, 3, 2, 4, 1)).astype(_BF16)

    def col(v):
        return np.ascontiguousarray(np.asarray(v, f32).reshape(KT, P).T)

    shared = {
        "Wi": Wi_bf, "Wo": Wo_bf,
        "w1R": conv_w(inputs["w1"]), "w2R": conv_w(inputs["w2"]),
        "TmD": TmD, "AfP": AfP_d, "AbP": AbP_d,
        "CfO": CfO.astype(_BF16), "CbO": CbO.astype(_BF16),
        "lamf": lamf, "lamb": lamb,
        "bc1c": col(inputs["bc1"]), "bc2c": col(inputs["bc2"]),
        "bi_row": _bf(inputs["bi"]).reshape(1, D),
        "bo_row": _bf(inputs["bo"]).reshape(1, D),
        "g1v": _bf(inputs["g1"]),
        "b1v": _bf(inputs["b1"]),
        "g2v": _bf(inputs["g2"]),
    }
    in_maps = []
    b2_fold = np.asarray(inputs["b2"], f32)[None, :]
    for b in range(n_cores):
        xb = np.ascontiguousarray(x[b])
        m = dict(shared)
        m["x_res"] = xb + b2_fold
        m["xT"] = np.ascontiguousarray(xb.T.astype(_BF16))
        m["nsc"] = np.ascontiguousarray(ns[b].reshape(KT, P).T)
        in_maps.append(m)
    return in_maps


def get_nc(L=_L, D=_D, n_cores=_B, debug_taps=False):
    key = (L, D, n_cores)
    if key not in _cache:
        _cache[key] = _build(L, D, n_cores)
    return _cache[key]


def kernel(**inputs):
    from concourse.bass_utils import run_bass_kernel_spmd

    L, D, B = _L, _D, _B
    nc = get_nc(L, D, B)
    in_maps = _prep_maps(inputs, L, D, B)
    res = run_bass_kernel_spmd(nc, in_maps, core_ids=list(range(B)))
    return np.stack([res.results[c]["out"] for c in range(B)]).astype(
        np.float32)


# revision 36
# speedup vs baseline: 1.0005x; 1.0005x over previous
"""Trainium2 Bass kernel for the DiffSSM block.

Data-parallel over batch B=8 across the 8 NeuronCores (one batch element
per core). All heavy compute runs on the TensorEngine in bf16 with fp32
PSUM accumulation; SSM kernel generation, timestep embedding, and the
Toeplitz diagonal block are host-side precompute.

Final design (cost-model device time 523 us vs 721 us for the phase-
serial baseline; PE-bound, TensorE busy 8 us -> 504 us of the span):

  - Phases: A proj-in+LN1 -> [xbar transposes] -> B SSD SSM mix ->
    D conv1+Silu -> E conv2 accumulate -> F proj-out+LN2+residual.
  - SSD decomposition of the bidirectional SSM global conv: one exact
    (chunk-invariant) 512x512 mixed Toeplitz diagonal block + rank-72
    cross-chunk state passing (64 modes + Df/Db const as a lambda=1
    state), with state recurrences fused into PSUM evictions.
  - LayerNorms: bias folded in as K=1 ones x bias_row matmuls, PSUM
    evicted immediately through ScalarE Copy into deep (8-buf) bf16
    tiles, stats + normalize + affine on VectorE at 2x bf16 throughput.
  - hln -> hlnT via 16 SBUF->SBUF xbar DMA transposes. The xbar mode
    switch serializes against ALL in-flight DMA copies, so a guard op
    (reads A's last tile, writes one byte into every transpose's output
    block) pins the whole batch to run back-to-back after A.
  - Queue partition: critical loads on HWDGE(sync); all latency-tolerant
    prefetches (weights, xT strips, Toeplitz block) as coalesced 3D-AP
    DMAs on SWDGE(gpsimd), double-buffered one phase ahead; transposes
    alone on HWDGE(scalar). Sqrt's 40k-entry activation table preloaded
    at kernel start.
  - Conv loops ot-outer with per-ot weight chunks (12 KB resident);
    pool lifetimes arranged so no release barrier blocks a queue.

Note: fp8/DoubleRow convs were tried and revert: measured rel err
3.9e-2 (> 2e-2 gate) because the conv path dominates output variance.
"""

import math

import numpy as np
import ml_dtypes

_BF16 = ml_dtypes.bfloat16

_L, _D, _B = 2048, 1024, 8

_cache = {}


def _build(L, D, n_cores):
    import concourse.bacc as bacc
    import concourse.bass as bass
    import concourse.tile as tile
    from concourse import mybir

    f32 = mybir.dt.float32
    bf16 = mybir.dt.bfloat16
    AF = mybir.ActivationFunctionType
    OP = mybir.AluOpType

    P = 128
    KT = D // P            # feature tiles
    LT = L // P            # sequence tiles
    ND = min(512, D)       # matmul free-dim chunk along features
    NF = min(512, L)       # matmul free-dim chunk along sequence
    EH = D // ND
    LC = L // NF
    ST = LT
    XSW = 256              # xT strip width
    SPL = XSW // P         # lt tiles per xT strip (2)
    TSW = 512              # transpose strip width (xbar free-dim mult 128)
    TPL = TSW // P         # lt tiles per transpose strip

    nc = bacc.Bacc("TRN2", target_bir_lowering=False, debug=False,
                   num_devices=n_cores)

    x_res = nc.dram_tensor("x_res", (L, D), f32, kind="ExternalInput").ap()
    xT = nc.dram_tensor("xT", (D, L), bf16, kind="ExternalInput").ap()
    Wi = nc.dram_tensor("Wi", (D, D), bf16, kind="ExternalInput").ap()
    w1R = nc.dram_tensor("w1R", (KT, P, KT, 3, P), bf16,
                         kind="ExternalInput").ap()
    w2R = nc.dram_tensor("w2R", (KT, P, KT, 3, P), bf16,
                         kind="ExternalInput").ap()
    Wo = nc.dram_tensor("Wo", (D, D), bf16, kind="ExternalInput").ap()
    NS = 72                # SSM states (64 modes + Df/Db const + pad)
    QC = 512               # SSD chunk length
    NCH = L // QC          # chunks
    SPC = QC // 128        # 128-tiles per chunk
    TmD = nc.dram_tensor("TmD", (1, 128, SPC, QC), bf16,
                         kind="ExternalInput").ap()
    AfP = nc.dram_tensor("AfP", (128, SPC, NS), bf16,
                         kind="ExternalInput").ap()
    AbP = nc.dram_tensor("AbP", (128, SPC, NS), bf16,
                         kind="ExternalInput").ap()
    CfO = nc.dram_tensor("CfO", (NS, QC), bf16, kind="ExternalInput").ap()
    CbO = nc.dram_tensor("CbO", (NS, QC), bf16, kind="ExternalInput").ap()
    lamf = nc.dram_tensor("lamf", (NS, 1), f32, kind="ExternalInput").ap()
    lamb = nc.dram_tensor("lamb", (NS, 1), f32, kind="ExternalInput").ap()
    nsc = nc.dram_tensor("nsc", (P, KT), f32, kind="ExternalInput").ap()
    bc1c = nc.dram_tensor("bc1c", (P, KT), f32, kind="ExternalInput").ap()
    bc2c = nc.dram_tensor("bc2c", (P, KT), f32, kind="ExternalInput").ap()
    bi_row = nc.dram_tensor("bi_row", (1, D), bf16, kind="ExternalInput").ap()
    bo_row = nc.dram_tensor("bo_row", (1, D), bf16, kind="ExternalInput").ap()
    vec_names = ["g1v", "b1v", "g2v"]
    vecs = {n: nc.dram_tensor(n, (D,), bf16, kind="ExternalInput").ap()
            for n in vec_names}
    out = nc.dram_tensor("out", (L, D), f32, kind="ExternalOutput").ap()

    with tile.TileContext(nc) as tc:
        # ---- pools (left stack, release order = reverse alloc) ----
        const = tc.alloc_tile_pool(name="const", bufs=1)
        statp = tc.alloc_tile_pool(name="stat", bufs=4)
        psum = tc.alloc_tile_pool(name="psum", bufs=8, space="PSUM")
        h2T_pool = tc.alloc_tile_pool(name="h2T", bufs=1)
        w2ch_pool = tc.alloc_tile_pool(name="w2ch", bufs=2)
        hlnT_pool = tc.alloc_tile_pool(name="hlnT", bufs=1)
        w1ch_pool = tc.alloc_tile_pool(name="w1ch", bufs=2)
        tb_pool = tc.alloc_tile_pool(name="tb", bufs=2)
        pa_pool = tc.alloc_tile_pool(name="pa", bufs=1)
        # right stack: hln (released end of B), then Fpool, co
        hln_pool = tc.alloc_tile_pool(name="hln", bufs=1, side="right")

        # ---- constants ----
        def rep_tile(name, pool=None):
            t = (pool or const).tile([P, D], bf16, tag=name,
                                     name=f"rep_{name}")
            ap = vecs[name]
            bcast = bass.AP(tensor=ap.tensor, offset=ap.offset,
                            ap=[[0, P]] + list(ap.ap))
            nc.gpsimd.dma_start(out=t[:], in_=bcast)
            return t

        bi_sb = const.tile([1, D], bf16)
        nc.sync.dma_start(out=bi_sb[:], in_=bi_row)
        # first xT strip on the sync queue, first Wi half on SWDGE, ahead
        # of all other const loads: the two queues run in parallel so the
        # first matmul group is fed ~4us sooner than a serial SWDGE chain.
        wi_sb = pa_pool.tile([P, KT, D], bf16, tag="wi")
        wi_r = Wi.rearrange("(kt p) d -> p kt d", p=P)
        xT_r = xT.rearrange("(kt p) l -> p kt l", p=P)
        xs0 = pa_pool.tile([P, KT, XSW], bf16, tag="xs", name="xs", bufs=4)
        nc.sync.dma_start(out=xs0[:], in_=xT_r[:, :, 0:XSW])
        nc.gpsimd.dma_start(out=wi_sb[:, :, 0:ND], in_=wi_r[:, :, 0:ND])
        eps_sb = const.tile([P, 1], f32)
        nc.vector.memset(eps_sb[:], 1e-5)
        ones_sb = const.tile([1, P], bf16)
        nc.vector.memset(ones_sb[:], 1.0)
        # preload the (large) sqrt activation table while initial DMAs run
        warm_sb = const.tile([P, 1], f32)
        nc.scalar.activation(out=warm_sb[:], in_=eps_sb[:], func=AF.Sqrt)
        ns_sb = const.tile([P, KT], f32)
        nc.sync.dma_start(out=ns_sb[:], in_=nsc)
        bc1_sb = const.tile([P, KT], f32)
        nc.sync.dma_start(out=bc1_sb[:], in_=bc1c)
        bc2_sb = const.tile([P, KT], f32)
        nc.sync.dma_start(out=bc2_sb[:], in_=bc2c)
        AfP_sb = const.tile([P, SPC, NS], bf16)
        nc.sync.dma_start(out=AfP_sb[:], in_=AfP)
        AbP_sb = const.tile([P, SPC, NS], bf16)
        nc.sync.dma_start(out=AbP_sb[:], in_=AbP)
        CfO_sb = const.tile([NS, QC], bf16)
        nc.sync.dma_start(out=CfO_sb[:], in_=CfO)
        CbO_sb = const.tile([NS, QC], bf16)
        nc.sync.dma_start(out=CbO_sb[:], in_=CbO)
        lamf_sb = const.tile([NS, 1], f32)
        nc.sync.dma_start(out=lamf_sb[:], in_=lamf)
        lamb_sb = const.tile([NS, 1], f32)
        nc.sync.dma_start(out=lamb_sb[:], in_=lamb)

        h2T_sb = h2T_pool.tile([P, KT, L], bf16)
        hlnT_sb = hlnT_pool.tile([P, KT, L], bf16)
        hln_sb = hln_pool.tile([P, LT, D], bf16)


        def ln_scalars(stats_tile):
            """stats -> (mv, rstd) tiles."""
            mv = statp.tile([P, 2], f32, tag="mv", name="mv")
            nc.vector.bn_aggr(out=mv[:], in_=stats_tile[:])
            std = statp.tile([P, 1], f32, tag="std", name="std")
            nc.scalar.activation(out=std[:], in_=mv[:, 1:2], func=AF.Sqrt,
                                 bias=eps_sb[:], scale=1.0)
            rstd = statp.tile([P, 1], f32, tag="rstd", name="rstd")
            nc.vector.reciprocal(out=rstd[:], in_=std[:])
            return mv, rstd

        # ---- Phase A: proj-in + LN1 (stats on PSUM, norm on ScalarE) ----
        xs = None
        for lt in range(LT):
            ls = lt // SPL
            if lt == 0:
                xs = xs0
            elif lt % SPL == 0:
                xs = pa_pool.tile([P, KT, XSW], bf16, tag="xs", name="xs",
                                  bufs=4)
                nc.gpsimd.dma_start(
                    out=xs[:],
                    in_=xT_r[:, :, ls * XSW:(ls + 1) * XSW])
            if lt == 0:
                nc.gpsimd.dma_start(out=wi_sb[:, :, ND:D],
                                    in_=wi_r[:, :, ND:D])
                g1_rep = rep_tile("g1v")
                b1_rep = rep_tile("b1v")
            if lt == 2:
                # the Toeplitz diag block is chunk-invariant: load it once
                Tc = tb_pool.tile([P, SPC, QC], bf16, tag="Tc", name="Tc",
                                  bufs=1)
                nc.gpsimd.dma_start(out=Tc[:], in_=TmD[0])
            col = (lt % SPL) * P
            stats = statp.tile([P, EH, 6], f32, tag="stats", name="stats")
            nrm = statp.tile([P, D], bf16, tag="nrm", name="nrm", bufs=8)
            for eh in range(EH):
                ps = psum.tile([P, ND], f32, tag="ps", name="ps")
                nc.tensor.matmul(ps[:], lhsT=ones_sb[:],
                                 rhs=bi_sb[:, eh * ND:(eh + 1) * ND],
                                 start=True, stop=False)
                for kt in range(KT):
                    nc.tensor.matmul(ps[:],
                                     lhsT=xs[:, kt, col:col + P],
                                     rhs=wi_sb[:, kt, eh * ND:(eh + 1) * ND],
                                     start=False, stop=(kt == KT - 1))
                nc.scalar.activation(out=nrm[:, eh * ND:(eh + 1) * ND],
                                     in_=ps[:], func=AF.Copy)
                nc.vector.bn_stats(out=stats[:, eh, :],
                                   in_=nrm[:, eh * ND:(eh + 1) * ND])
            mv, rstd = ln_scalars(stats)
            nc.vector.tensor_scalar(out=nrm[:], in0=nrm[:],
                                    scalar1=mv[:, 0:1], scalar2=rstd[:],
                                    op0=OP.subtract, op1=OP.mult)
            nc.vector.tensor_mul(out=nrm[:], in0=nrm[:], in1=g1_rep[:])
            nc.vector.tensor_add(out=hln_sb[:, lt, :], in0=nrm[:],
                                 in1=b1_rep[:])
        # hln -> hlnT SBUF->SBUF xbar transposes, one per lt tile. The
        # xbar-mode switch serializes against ALL in-flight DMA copies, so
        # a transpose scheduled mid-A stalls the ACT sequencer for ~10us.
        # Guard: write one byte into every transpose's output block, with
        # the guard reading A's last hln tile -- every transpose then
        # WAW-depends on A being fully done and the batch runs back-to-back
        # at B's start (gating only phase D).
        hlnT_r = hlnT_sb[:].rearrange("p kt (lt c) -> p kt lt c", c=P)
        nc.scalar.activation(out=hlnT_r[:, 0, :, 0:1],
                             in_=hln_sb[:, LT - 1, 0:LT], func=AF.Copy)
        for lt in range(LT):
            nc.scalar.dma_start_transpose(
                out=hlnT_sb[:, :, lt * P:(lt + 1) * P],
                in_=hln_sb[:, lt, :])

        # ---- Phase B: SSD chunked SSM mix ----
        # cross-chunk states: Zf[c] = sum_{cs<c} Lam^(Q(c-cs-1)) Pf[cs],
        # Gb[c] = sum_{cs>c} Lam^(Q(cs-c-1)) Pb[cs]; recurrences fused into
        # the PSUM evictions (scalar_tensor_tensor).
        def state_proj(proj_sb, c):
            pss = []
            for eh in range(EH):
                ps = psum.tile([NS, ND], f32, tag="ps", name="psP")
                for st in range(SPC):
                    nc.tensor.matmul(
                        ps[:], lhsT=proj_sb[:, st, :],
                        rhs=hln_sb[:, c * SPC + st, eh * ND:(eh + 1) * ND],
                        start=(st == 0), stop=(st == SPC - 1))
                pss.append(ps)
            return pss

        def state_tile(name):
            return tb_pool.tile([NS, D], bf16, tag=name, name=name, bufs=1)

        Zf = {}
        for c in range(NCH - 1):
            pss = state_proj(AfP_sb, c)
            Zf[c + 1] = state_tile(f"Zf{c + 1}")
            for eh in range(EH):
                sl = slice(eh * ND, (eh + 1) * ND)
                if c == 0:
                    nc.vector.tensor_copy(out=Zf[1][:, sl],
                                          in_=pss[eh][:])
                else:
                    nc.vector.scalar_tensor_tensor(
                        out=Zf[c + 1][:, sl], in0=Zf[c][:, sl],
                        scalar=lamf_sb[:, 0:1], in1=pss[eh][:],
                        op0=OP.mult, op1=OP.add)
        Gb = {}
        for c in range(NCH - 1, 0, -1):
            pss = state_proj(AbP_sb, c)
            Gb[c - 1] = state_tile(f"Gb{c - 1}")
            for eh in range(EH):
                sl = slice(eh * ND, (eh + 1) * ND)
                if c == NCH - 1:
                    nc.vector.tensor_copy(out=Gb[c - 1][:, sl],
                                          in_=pss[eh][:])
                else:
                    nc.vector.scalar_tensor_tensor(
                        out=Gb[c - 1][:, sl], in0=Gb[c][:, sl],
                        scalar=lamb_sb[:, 0:1], in1=pss[eh][:],
                        op0=OP.mult, op1=OP.add)

        w1_next = None
        for tch in range(NCH):
            if tch == 0:
                # prefetch first conv1 weight chunk during B
                w1_next = w1ch_pool.tile([P, KT, 3, P], bf16, tag="w1c",
                                         name="w1c")
                nc.gpsimd.dma_start(out=w1_next[:], in_=w1R[0])
            for dt in range(KT):
                ps = psum.tile([P, QC], f32, tag="ps", name="ps")
                n_mm = SPC + (tch > 0) + (tch < NCH - 1)
                k = 0
                for st in range(SPC):
                    k += 1
                    nc.tensor.matmul(
                        ps[:],
                        lhsT=hln_sb[:, tch * SPC + st, dt * P:(dt + 1) * P],
                        rhs=Tc[:, st, :],
                        start=(st == 0), stop=(k == n_mm))
                if tch > 0:
                    k += 1
                    nc.tensor.matmul(ps[:],
                                     lhsT=Zf[tch][:, dt * P:(dt + 1) * P],
                                     rhs=CfO_sb[:], start=False,
                                     stop=(k == n_mm))
                if tch < NCH - 1:
                    k += 1
                    nc.tensor.matmul(ps[:],
                                     lhsT=Gb[tch][:, dt * P:(dt + 1) * P],
                                     rhs=CbO_sb[:], start=False,
                                     stop=(k == n_mm))
                nc.vector.tensor_scalar_mul(
                    out=h2T_sb[:, dt, tch * QC:(tch + 1) * QC],
                    in0=ps[:], scalar1=ns_sb[:, dt:dt + 1])
        pa_pool.release()
        tb_pool.release()
        hln_pool.release()

        # Fpool + co on the (now empty) right stack; loads overlap D/E.
        Fpool = tc.alloc_tile_pool(name="Fp", bufs=1, side="right")
        co_pool = tc.alloc_tile_pool(name="co", bufs=1, side="right")
        wo_sb = Fpool.tile([P, KT, D], bf16, tag="wo")
        wo_r = Wo.rearrange("(dt p) e -> dt p e", p=P)
        for dt in range(KT):
            nc.gpsimd.dma_start(out=wo_sb[:, dt, :], in_=wo_r[dt])
        bo_sb = Fpool.tile([1, D], bf16, tag="bo")
        nc.sync.dma_start(out=bo_sb[:], in_=bo_row)
        g2_rep = rep_tile("g2v", pool=Fpool)
        co_sb = co_pool.tile([P, KT, L], bf16)

        def conv_mms(ps, w_t, src_sb, lc):
            # kernel-3 conv as 3 shifted matmuls; j=1 (no shift) first so
            # start=True initializes the whole PSUM range; border columns
            # handled by narrowing the edge matmuls.
            first = True
            for it in range(KT):
                for j in (1, 0, 2):
                    o0 = 1 if (j == 0 and lc == 0) else 0
                    o1 = NF - 1 if (j == 2 and lc == LC - 1) else NF
                    base = lc * NF + j - 1
                    nc.tensor.matmul(
                        ps[:, o0:o1],
                        lhsT=w_t[:, it, j, :],
                        rhs=src_sb[:, it, base + o0:base + o1],
                        start=first,
                        stop=(it == KT - 1 and j == 2))
                    first = False

        # ---- Phase D: conv1 (+Silu), ot-outer with weight streaming ----
        w2_next = None
        for ot in range(KT):
            w1t = w1_next
            if ot + 1 < KT:
                w1_next = w1ch_pool.tile([P, KT, 3, P], bf16, tag="w1c",
                                         name="w1c")
                nc.gpsimd.dma_start(out=w1_next[:], in_=w1R[ot + 1])
            if ot == 0:
                w2_next = w2ch_pool.tile([P, KT, 3, P], bf16, tag="w2c",
                                         name="w2c")
                nc.gpsimd.dma_start(out=w2_next[:], in_=w2R[0])
            for lc in range(LC):
                ps = psum.tile([P, NF], f32, tag="ps", name="ps")
                conv_mms(ps, w1t, hlnT_sb, lc)
                nc.scalar.activation(
                    out=co_sb[:, ot, lc * NF:(lc + 1) * NF],
                    in_=ps[:], func=AF.Silu, bias=bc1_sb[:, ot:ot + 1],
                    scale=1.0)
        w1ch_pool.release()
        hlnT_pool.release()

        # ---- Phase E: conv2, accumulate into h2T ----
        for ot in range(KT):
            w2t = w2_next
            if ot + 1 < KT:
                w2_next = w2ch_pool.tile([P, KT, 3, P], bf16, tag="w2c",
                                         name="w2c")
                nc.gpsimd.dma_start(out=w2_next[:], in_=w2R[ot + 1])
            for lc in range(LC):
                ps = psum.tile([P, NF], f32, tag="ps", name="ps")
                conv_mms(ps, w2t, co_sb, lc)
                nc.vector.scalar_tensor_tensor(
                    out=h2T_sb[:, ot, lc * NF:(lc + 1) * NF],
                    in0=ps[:], scalar=bc2_sb[:, ot:ot + 1],
                    in1=h2T_sb[:, ot, lc * NF:(lc + 1) * NF],
                    op0=OP.add, op1=OP.add)
        w2ch_pool.release()
        co_pool.release()

        # ---- Phase F: proj-out + LN2 + residual ----
        x_r = x_res.rearrange("(t p) d -> t p d", p=P)
        out_r = out.rearrange("(t p) d -> t p d", p=P)
        for lt in range(LT):
            x_t = Fpool.tile([P, D], f32, tag="x_t", name="x_t", bufs=2)
            nc.sync.dma_start(out=x_t[:], in_=x_r[lt])
            stats = statp.tile([P, EH, 6], f32, tag="stats", name="stats")
            y = Fpool.tile([P, D], bf16, tag="y", name="y", bufs=4)
            for eh in range(EH):
                ps = psum.tile([P, ND], f32, tag="ps", name="ps")
                nc.tensor.matmul(ps[:], lhsT=ones_sb[:],
                                 rhs=bo_sb[:, eh * ND:(eh + 1) * ND],
                                 start=True, stop=False)
                for dt in range(KT):
                    nc.tensor.matmul(ps[:],
                                     lhsT=h2T_sb[:, dt, lt * P:(lt + 1) * P],
                                     rhs=wo_sb[:, dt, eh * ND:(eh + 1) * ND],
                                     start=False, stop=(dt == KT - 1))
                nc.scalar.activation(out=y[:, eh * ND:(eh + 1) * ND],
                                     in_=ps[:], func=AF.Copy)
                nc.vector.bn_stats(out=stats[:, eh, :],
                                   in_=y[:, eh * ND:(eh + 1) * ND])
            mv, rstd = ln_scalars(stats)
            nc.vector.tensor_scalar(out=y[:], in0=y[:],
                                    scalar1=mv[:, 0:1], scalar2=rstd[:],
                                    op0=OP.subtract, op1=OP.mult)
            nc.vector.tensor_mul(out=y[:], in0=y[:], in1=g2_rep[:])
            nc.vector.tensor_add(out=x_t[:], in0=y[:], in1=x_t[:])
            nc.sync.dma_start(out=out_r[lt], in_=x_t[:])
        h2T_pool.release()
        psum.release()
        statp.release()
        const.release()
        Fpool.release()

    nc.compile()
    return nc


def _bf(a):
    return np.ascontiguousarray(np.asarray(a, np.float32)).astype(_BF16)


def _prep_maps(inputs, L, D, n_cores):
    P = 128
    KT = D // P
    f32 = np.float32
    x = np.asarray(inputs["x"], f32)
    t = np.asarray(inputs["t"], f32)
    beta1 = float(np.asarray(inputs["beta1"], f32)[0])
    beta2 = float(np.asarray(inputs["beta2"], f32)[0])

    # SSM kernels -> mixed Toeplitz (transposed), host fp32
    af = np.diagonal(np.asarray(inputs["Af"], f32))
    ab = np.diagonal(np.asarray(inputs["Ab"], f32))
    l_ar = np.arange(L, dtype=f32)[:, None]
    kf = np.exp(l_ar * af[None, :]) @ (
        np.asarray(inputs["Bf"], f32)[:, 0] * np.asarray(inputs["Cf"], f32)[0]
    ) + np.asarray(inputs["Df"], f32)[0]
    kb = np.exp(l_ar * ab[None, :]) @ (
        np.asarray(inputs["Bb"], f32)[:, 0] * np.asarray(inputs["Cb"], f32)[0]
    ) + np.asarray(inputs["Db"], f32)[0]
    # within-chunk mixed Toeplitz diagonal blocks (exact)
    QC, NS = 512, 72
    NCH = L // QC
    tms = np.arange(QC)[None, :] - np.arange(QC)[:, None]  # [s_loc, t_loc]
    TmQ = (np.where(tms >= 0, beta1 * kf[np.clip(tms, 0, None)], 0.0)
           + np.where(tms <= 0, beta2 * kb[np.clip(-tms, 0, None)], 0.0))
    TmD = np.ascontiguousarray(
        TmQ.reshape(1, QC // 128, 128, QC).transpose(0, 2, 1, 3)
    ).astype(f32).astype(_BF16)
    # cross-chunk rank-NS state matrices (64 modes + const Df/Db state)
    wf = (np.asarray(inputs["Bf"], f32)[:, 0]
          * np.asarray(inputs["Cf"], f32)[0])
    wb = (np.asarray(inputs["Bb"], f32)[:, 0]
          * np.asarray(inputs["Cb"], f32)[0])
    Df = float(np.asarray(inputs["Df"], f32)[0])
    Db = float(np.asarray(inputs["Db"], f32)[0])
    s_loc = np.arange(QC, dtype=f32)
    AfP = np.zeros((QC, NS), f32)
    AfP[:, :64] = np.exp((QC - 1 - s_loc)[:, None] * af[None, :])
    AfP[:, 64] = 1.0
    AbP = np.zeros((QC, NS), f32)
    AbP[:, :64] = np.exp((s_loc + 1)[:, None] * ab[None, :])
    AbP[:, 64] = 1.0
    t_loc = np.arange(QC, dtype=f32)
    CfO = np.zeros((NS, QC), f32)
    CfO[:64] = beta1 * wf[:, None] * np.exp(af[:, None] * (t_loc + 1)[None])
    CfO[64] = beta1 * Df
    CbO = np.zeros((NS, QC), f32)
    CbO[:64] = beta2 * wb[:, None] * np.exp(
        ab[:, None] * (QC - 1 - t_loc)[None])
    CbO[64] = beta2 * Db
    lamf = np.zeros((NS, 1), f32)
    lamf[:64, 0] = np.exp(af * QC)
    lamf[64, 0] = 1.0
    lamb = np.zeros((NS, 1), f32)
    lamb[:64, 0] = np.exp(ab * QC)
    lamb[64, 0] = 1.0
    AfP_d = np.ascontiguousarray(
        AfP.reshape(QC // 128, 128, NS).transpose(1, 0, 2)).astype(_BF16)
    AbP_d = np.ascontiguousarray(
        AbP.reshape(QC // 128, 128, NS).transpose(1, 0, 2)).astype(_BF16)

    # timestep embedding -> noise scale (B, D)
    half = D // 2
    freqs = np.exp(np.arange(half, dtype=f32)
                   * (-math.log(10000.0) / (half - 1)))
    ang = t[:, None] * freqs[None, :]
    emb = np.concatenate([np.sin(ang), np.cos(ang)], axis=1).astype(f32)
    ns = (1.0 / (1.0 + np.exp(-emb))).astype(f32)         # (B, D)

    Wi_bf = _bf(inputs["Wi"])
    Wo_bf = _bf(inputs["Wo"])

    def conv_w(w):
        # (D_o, D_i, 3) -> [ot, p_i, it, j, o_local]
        w = np.asarray(w, f32).reshape(KT, P, KT, P, 3)
        return np.ascontiguousarray(
            w.transpose(0, 3, 2, 4, 1)).astype(_BF16)

    def col(v):
        return np.ascontiguousarray(np.asarray(v, f32).reshape(KT, P).T)

    shared = {
        "Wi": Wi_bf, "Wo": Wo_bf,
        "w1R": conv_w(inputs["w1"]), "w2R": conv_w(inputs["w2"]),
        "TmD": TmD, "AfP": AfP_d, "AbP": AbP_d,
        "CfO": CfO.astype(_BF16), "CbO": CbO.astype(_BF16),
        "lamf": lamf, "lamb": lamb,
        "bc1c": col(inputs["bc1"]), "bc2c": col(inputs["bc2"]),
        "bi_row": _bf(inputs["bi"]).reshape(1, D),
        "bo_row": _bf(inputs["bo"]).reshape(1, D),
        "g1v": _bf(inputs["g1"]),
        "b1v": _bf(inputs["b1"]),
        "g2v": _bf(inputs["g2"]),
    }
    in_maps = []
    b2_fold = np.asarray(inputs["b2"], f32)[None, :]
    for b in range(n_cores):
        xb = np.ascontiguousarray(x[b])
        m = dict(shared)
        m["x_res"] = xb + b2_fold
        m["xT"] = np.ascontiguousarray(xb.T.astype(_BF16))
        m["nsc"] = np.ascontiguousarray(ns[b].reshape(KT, P).T)
        in_maps.append(m)
    return in_maps


def get_nc(L=_L, D=_D, n_cores=_B, debug_taps=False):
    key = (L, D, n_cores)
    if key not in _cache:
        _cache[key] = _build(L, D, n_cores)
    return _cache[key]


def kernel(**inputs):
    from concourse.bass_utils import run_bass_kernel_spmd

    L, D, B = _L, _D, _B
    nc = get_nc(L, D, B)
    in_maps = _prep_maps(inputs, L, D, B)
    res = run_bass_kernel_spmd(nc, in_maps, core_ids=list(range(B)))
    return np.stack([res.results[c]["out"] for c in range(B)]).astype(
        np.float32)
